# revision 23
# baseline (speedup 1.0000x reference)
"""Trainium2 Bass kernel for nn_AttentionMapLayer.

Computes out[b,h,w,c] = (l2n(s_o)[b,w] * l2n(t_o)[b,h] + roi[h,w]) * ipt[b,h,w,c]
where l2n is tf-style l2_normalize (x * rsqrt(max(sum(x^2), 1e-12))).

Sharding: pure data parallel over batch (16) across 8 NeuronCores, 2 batches
per core; roi_map replicated. Per core the kernel is HBM-bandwidth bound:
~30.7 MB read (ipt shard) + ~30.7 MB written (out shard).

Per-core structure (v3):
  - (b, h) flattened to 600 rows; ipt/out declared as [600, 25, 512] and
    roi_map replicated host-side to [600, 25], so stream tiles use the full
    128 partitions (=> all 16 SDMA engines) and prologue loads are one DMA
    per row-tile (no batch-boundary segment DMAs except the s broadcast).
  - l2-normalization factors on 1-partition tiles; both rsqrt factors folded
    into s: s_hat = s_o * rs_s * rs_t, so a = s_hat (outer) t_o_raw + roi.
    (A K=1 TensorE matmul outer product wedges the device; tensor_tensor_
    reduce also wedges it — both avoided.)
  - s_hat broadcast across partitions by a partition-stride-0 DMA read from
    a DRAM scratch; t_o loaded column-wise (rows on partitions).
  - a/roi/t/s tiles are separate per row-tile so the Tile framework's
    per-tensor semaphores let row-tile 0's stream start as soon as ITS
    attention row is ready, not after the whole prologue.
  - main stream: row tiles of 128|88 partitions x 9|8-w chunks; DMA in on
    SyncE HWDGE queue (SyncE issues nothing else, so ins start at t=0),
    multiply into a separate out tile, DMA out on ScalarE HWDGE queue
    (ScalarE also issues the small prologue DMAs, which it finishes before
    the first out is ready).
"""

import os
import sys

import numpy as np

for _p in (
    "/root/.axon_site",
    "/root/.axon_site/_ro/trn_rl_repo",
    "/root/.axon_site/_ro/pypackages",
    "/opt/trn_rl_repo",
):
    if os.path.isdir(_p) and _p not in sys.path:
        sys.path.append(_p)

import concourse.bacc as bacc
import concourse.bass as bass
import concourse.tile as tile
from concourse import mybir
from concourse.bass_utils import run_bass_kernel_spmd

N_CORES = 8
B, H, W, C = 16, 300, 25, 512
NB = B // N_CORES   # batches per core
NR = NB * H         # flattened rows per core
ROW_TILES = ((0, 128), (128, 128), (512, 88), (256, 128), (384, 128))
W_SPLITS = ((0, 7), (7, 13), (13, 19), (19, 25))
EPS = 1e-12

_NC_CACHE = []


def _segments(r0, plen):
    """Split rows [r0, r0+plen) at batch boundaries -> (p0, b, h0, seglen)."""
    segs = []
    r = r0
    while r < r0 + plen:
        b, h0 = divmod(r, H)
        seglen = min(r0 + plen - r, H - h0)
        segs.append((r - r0, b, h0, seglen))
        r += seglen
    return segs


def _build():
    dt = mybir.dt.float32
    nc = bacc.Bacc(None)
    s_o = nc.declare_dram_parameter("s_o", [NB, W], dt, isOutput=False)
    t_o = nc.declare_dram_parameter("t_o", [NB, H], dt, isOutput=False)
    ipt = nc.declare_dram_parameter("ipt", [NR, W, C], dt, isOutput=False)
    roi = nc.declare_dram_parameter("roi_map", [NR, W], dt, isOutput=False)
    out = nc.declare_dram_parameter("out", [NR, W, C], dt, isOutput=True)

    t_flat = t_o.rearrange("b h -> (b h)")
    mult = mybir.AluOpType.mult
    NT = len(ROW_TILES)

    with tile.TileContext(nc) as tc:
        with (
            tc.tile_pool(name="small", bufs=1) as small,
            tc.tile_pool(name="dram", bufs=1, space="DRAM") as dram,
            tc.tile_pool(name="big", bufs=8) as big,
            tc.tile_pool(name="bigo", bufs=4) as bigo,
        ):
            s_hat_d = dram.tile([NB, W], dt)
            # per-batch 1-partition tiles for the normalization factors
            s_sb = [small.tile([1, W], dt, name=f"s{b}", tag=f"s{b}") for b in range(NB)]
            t_sb = [small.tile([1, H], dt, name=f"t{b}", tag=f"t{b}") for b in range(NB)]
            sq_s = [small.tile([1, W], dt, name=f"qs{b}", tag=f"qs{b}") for b in range(NB)]
            sq_t = [small.tile([1, H], dt, name=f"qt{b}", tag=f"qt{b}") for b in range(NB)]
            rs_s = [small.tile([1, 1], dt, name=f"rs{b}", tag=f"rs{b}") for b in range(NB)]
            rs_t = [small.tile([1, 1], dt, name=f"rt{b}", tag=f"rt{b}") for b in range(NB)]
            # per-row-tile tiles (separate tensors -> fine-grained semaphores)
            t_col = [small.tile([128, 1], dt, name=f"tc{i}", tag=f"tc{i}") for i in range(NT)]
            roi_sb = [small.tile([128, W], dt, name=f"ro{i}", tag=f"ro{i}") for i in range(NT)]
            s_row = [small.tile([128, W], dt, name=f"sr{i}", tag=f"sr{i}") for i in range(NT)]
            a_sb = [small.tile([128, W], dt, name=f"a{i}", tag=f"a{i}") for i in range(NT)]

            # early chunk-1 stream ins on ScalarE queue: the SyncE queue
            # alone caps ~250 GB/s, so the ramp needs a second queue before
            # the outs exist. Allocate those tiles up front.
            early_in = {}
            for rt, (r0, plen) in enumerate(ROW_TILES):
                t1 = big.tile([128, 7, C], dt, name="stream", tag="stream")
                w0, w1 = W_SPLITS[1]
                nc.scalar.dma_start(
                    out=t1[:plen, : w1 - w0, :], in_=ipt[r0 : r0 + plen, w0:w1, :]
                )
                early_in[rt] = t1

            # ---- prologue loads ----
            # rt0-critical chain on ScalarE HWDGE (fast issue; SyncE kept
            # free for stream ins); rt1.. loads on GpSimd (parallel issuer).
            for b in range(NB):
                nc.scalar.dma_start(out=s_sb[b][:], in_=s_o[b : b + 1, :])
                nc.scalar.dma_start(out=t_sb[b][:], in_=t_o[b : b + 1, :])
            for rt, (r0, plen) in enumerate(ROW_TILES):
                nc.scalar.dma_start(
                    out=t_col[rt][:plen, :], in_=t_flat[r0 : r0 + plen]
                )
                nc.scalar.dma_start(
                    out=roi_sb[rt][:plen, :], in_=roi[r0 : r0 + plen, :]
                )

            # rs = 1/sqrt(max(sum(x^2), eps)) per vector; fold both into s:
            # s_hat = s_o * rs_s * rs_t  (so a = s_hat (outer) t_o + roi)
            for b in range(NB):
                for sq, sb, rs in (
                    (sq_s[b], s_sb[b], rs_s[b]),
                    (sq_t[b], t_sb[b], rs_t[b]),
                ):
                    nc.vector.tensor_mul(out=sq[:], in0=sb[:], in1=sb[:])
                    nc.vector.reduce_sum(
                        out=rs[:], in_=sq[:], axis=mybir.AxisListType.X
                    )
                    nc.vector.tensor_scalar_max(out=rs[:], in0=rs[:], scalar1=EPS)
                    nc.scalar.sqrt(out=rs[:], in_=rs[:])
                    nc.vector.reciprocal(out=rs[:], in_=rs[:])
                nc.vector.tensor_scalar(
                    out=s_sb[b][:], in0=s_sb[b][:], scalar1=rs_s[b][:],
                    scalar2=rs_t[b][:], op0=mult, op1=mult,
                )
                nc.scalar.dma_start(out=s_hat_d[b : b + 1, :], in_=s_sb[b][:])

            # s_row[rt][p, :] = s_hat[b(row)] via partition-stride-0 DMA bcast
            for rt, (r0, plen) in enumerate(ROW_TILES):
                for p0, b, h0, seglen in _segments(r0, plen):
                    base = s_hat_d[b, :]
                    bcast = bass.AP(
                        tensor=base.tensor,
                        offset=base.offset,
                        ap=[[0, seglen]] + list(base.ap),
                    )
                    nc.scalar.dma_start(
                        out=s_row[rt][p0 : p0 + seglen, :], in_=bcast
                    )

            # a[rt] = s_row * t_col + roi (full-width DVE, start partition 0)
            for rt, (r0, plen) in enumerate(ROW_TILES):
                nc.vector.tensor_scalar_mul(
                    out=a_sb[rt][:plen, :], in0=s_row[rt][:plen, :],
                    scalar1=t_col[rt][:plen, :],
                )
                nc.vector.tensor_add(
                    out=a_sb[rt][:plen, :], in0=a_sb[rt][:plen, :],
                    in1=roi_sb[rt][:plen, :],
                )

            # ---- main bandwidth-bound stream ----
            # chunk-1 ins were issued early on ScalarE; their outs go on the
            # SyncE queue (after all its ins) so both queues carry ~equal
            # bytes and the drain tail runs two-queue.
            sync_outs = []
            for rt, (r0, plen) in enumerate(ROW_TILES):
                for ci, (w0, w1) in enumerate(W_SPLITS):
                    nw = w1 - w0
                    if ci == 1:
                        t = early_in[rt]
                    else:
                        t = big.tile([128, 7, C], dt, name="stream", tag="stream")
                        nc.sync.dma_start(
                            out=t[:plen, :nw, :], in_=ipt[r0 : r0 + plen, w0:w1, :]
                        )
                    to = bigo.tile([128, 7, C], dt, name="ostream", tag="ostream")
                    for wi in range(nw):
                        nc.vector.tensor_scalar_mul(
                            out=to[:plen, wi, :],
                            in0=t[:plen, wi, :],
                            scalar1=a_sb[rt][:plen, w0 + wi : w0 + wi + 1],
                        )
                    if ci == 1:
                        sync_outs.append((r0, plen, w0, w1, to))
                    else:
                        nc.scalar.dma_start(
                            out=out[r0 : r0 + plen, w0:w1, :], in_=to[:plen, :nw, :]
                        )
            for r0, plen, w0, w1, to in sync_outs:
                nc.sync.dma_start(
                    out=out[r0 : r0 + plen, w0:w1, :], in_=to[:plen, : w1 - w0, :]
                )
    nc.finalize()
    return nc


def _get_nc():
    if not _NC_CACHE:
        _NC_CACHE.append(_build())
    return _NC_CACHE[0]


def _make_in_maps(s_o, t_o, ipt, roi_map):
    s_o = np.ascontiguousarray(np.asarray(s_o, dtype=np.float32))
    t_o = np.ascontiguousarray(np.asarray(t_o, dtype=np.float32))
    ipt = np.asarray(ipt, dtype=np.float32)
    roi_map = np.asarray(roi_map, dtype=np.float32)
    roi_rep = np.ascontiguousarray(
        np.broadcast_to(roi_map.reshape(1, H, W), (NB, H, W)).reshape(NR, W)
    )
    in_maps = []
    for i in range(N_CORES):
        lo, hi = i * NB, (i + 1) * NB
        in_maps.append(
            {
                "s_o": s_o[lo:hi],
                "t_o": t_o[lo:hi],
                "ipt": np.ascontiguousarray(ipt[lo:hi]).reshape(NR, W, C),
                "roi_map": roi_rep,
            }
        )
    return in_maps


def _execute(in_maps, **kwargs):
    nc = _get_nc()
    return run_bass_kernel_spmd(nc, in_maps, core_ids=list(range(N_CORES)), **kwargs)


def kernel(s_o, t_o, ipt, roi_map):
    in_maps = _make_in_maps(s_o, t_o, ipt, roi_map)
    res = _execute(in_maps)
    return np.concatenate(
        [res.results[i]["out"].reshape(NB, H, W, C) for i in range(N_CORES)], axis=0
    )


# revision 24
# speedup vs baseline: 1.0678x; 1.0678x over previous
"""Trainium2 Bass kernel for nn_AttentionMapLayer.

Computes out[b,h,w,c] = (l2n(s_o)[b,w] * l2n(t_o)[b,h] + roi[h,w]) * ipt[b,h,w,c]
where l2n is tf-style l2_normalize (x * rsqrt(max(sum(x^2), 1e-12))).

Sharding: pure data parallel over batch (16) across 8 NeuronCores, 2 batches
per core; roi_map replicated. Per core the kernel is HBM-bandwidth bound:
~30.7 MB read (ipt shard) + ~30.7 MB written (out shard).

Per-core structure (v3):
  - (b, h) flattened to 600 rows; ipt/out declared as [600, 25, 512] and
    roi_map replicated host-side to [600, 25], so stream tiles use the full
    128 partitions (=> all 16 SDMA engines) and prologue loads are one DMA
    per row-tile (no batch-boundary segment DMAs except the s broadcast).
  - l2-normalization factors on 1-partition tiles; both rsqrt factors folded
    into s: s_hat = s_o * rs_s * rs_t, so a = s_hat (outer) t_o_raw + roi.
    (A K=1 TensorE matmul outer product wedges the device; tensor_tensor_
    reduce also wedges it — both avoided.)
  - s_hat broadcast across partitions by a partition-stride-0 DMA read from
    a DRAM scratch; t_o loaded column-wise (rows on partitions).
  - a/roi/t/s tiles are separate per row-tile so the Tile framework's
    per-tensor semaphores let row-tile 0's stream start as soon as ITS
    attention row is ready, not after the whole prologue.
  - main stream: row tiles of 128|88 partitions x 9|8-w chunks; DMA in on
    SyncE HWDGE queue (SyncE issues nothing else, so ins start at t=0),
    multiply into a separate out tile, DMA out on ScalarE HWDGE queue
    (ScalarE also issues the small prologue DMAs, which it finishes before
    the first out is ready).
"""

import os
import sys

import numpy as np

for _p in (
    "/root/.axon_site",
    "/root/.axon_site/_ro/trn_rl_repo",
    "/root/.axon_site/_ro/pypackages",
    "/opt/trn_rl_repo",
):
    if os.path.isdir(_p) and _p not in sys.path:
        sys.path.append(_p)

import concourse.bacc as bacc
import concourse.bass as bass
import concourse.tile as tile
from concourse import mybir
from concourse.bass_utils import run_bass_kernel_spmd

N_CORES = 8
B, H, W, C = 16, 300, 25, 512
NB = B // N_CORES   # batches per core
NR = NB * H         # flattened rows per core
ROW_TILES = ((0, 128), (128, 128), (512, 88), (256, 128), (384, 128))
W_SPLITS = ((0, 7), (7, 13), (13, 19), (19, 25))
EPS = 1e-12

_NC_CACHE = []


def _segments(r0, plen):
    """Split rows [r0, r0+plen) at batch boundaries -> (p0, b, h0, seglen)."""
    segs = []
    r = r0
    while r < r0 + plen:
        b, h0 = divmod(r, H)
        seglen = min(r0 + plen - r, H - h0)
        segs.append((r - r0, b, h0, seglen))
        r += seglen
    return segs


def _build():
    dt = mybir.dt.float32
    nc = bacc.Bacc(None)
    s_o = nc.declare_dram_parameter("s_o", [NB, W], dt, isOutput=False)
    t_o = nc.declare_dram_parameter("t_o", [NB, H], dt, isOutput=False)
    ipt = nc.declare_dram_parameter("ipt", [NR, W, C], dt, isOutput=False)
    roi = nc.declare_dram_parameter("roi_map", [NR, W], dt, isOutput=False)
    out = nc.declare_dram_parameter("out", [NR, W, C], dt, isOutput=True)

    t_flat = t_o.rearrange("b h -> (b h)")
    mult = mybir.AluOpType.mult
    NT = len(ROW_TILES)

    with tile.TileContext(nc) as tc:
        with (
            tc.tile_pool(name="small", bufs=1) as small,
            tc.tile_pool(name="dram", bufs=1, space="DRAM") as dram,
            tc.tile_pool(name="big", bufs=8) as big,
            tc.tile_pool(name="bigo", bufs=4) as bigo,
        ):
            s_hat_d = dram.tile([NB, W], dt)
            # per-batch 1-partition tiles for the normalization factors
            s_sb = [small.tile([1, W], dt, name=f"s{b}", tag=f"s{b}") for b in range(NB)]
            t_sb = [small.tile([1, H], dt, name=f"t{b}", tag=f"t{b}") for b in range(NB)]
            sq_s = [small.tile([1, W], dt, name=f"qs{b}", tag=f"qs{b}") for b in range(NB)]
            sq_t = [small.tile([1, H], dt, name=f"qt{b}", tag=f"qt{b}") for b in range(NB)]
            rs_s = [small.tile([1, 1], dt, name=f"rs{b}", tag=f"rs{b}") for b in range(NB)]
            rs_t = [small.tile([1, 1], dt, name=f"rt{b}", tag=f"rt{b}") for b in range(NB)]
            # per-row-tile tiles (separate tensors -> fine-grained semaphores)
            t_col = [small.tile([128, 1], dt, name=f"tc{i}", tag=f"tc{i}") for i in range(NT)]
            roi_sb = [small.tile([128, W], dt, name=f"ro{i}", tag=f"ro{i}") for i in range(NT)]
            s_row = [small.tile([128, W], dt, name=f"sr{i}", tag=f"sr{i}") for i in range(NT)]
            a_sb = [small.tile([128, W], dt, name=f"a{i}", tag=f"a{i}") for i in range(NT)]

            # ---- prologue loads ----
            # rt0-critical chain on ScalarE HWDGE (fast issue; SyncE kept
            # free for stream ins); rt1.. loads on GpSimd (parallel issuer).
            for b in range(NB):
                nc.scalar.dma_start(out=s_sb[b][:], in_=s_o[b : b + 1, :])
                nc.scalar.dma_start(out=t_sb[b][:], in_=t_o[b : b + 1, :])
            for rt, (r0, plen) in enumerate(ROW_TILES):
                nc.scalar.dma_start(
                    out=t_col[rt][:plen, :], in_=t_flat[r0 : r0 + plen]
                )
                nc.scalar.dma_start(
                    out=roi_sb[rt][:plen, :], in_=roi[r0 : r0 + plen, :]
                )

            # rs = 1/sqrt(max(sum(x^2), eps)) per vector; fold both into s:
            # s_hat = s_o * rs_s * rs_t  (so a = s_hat (outer) t_o + roi)
            for b in range(NB):
                for sq, sb, rs in (
                    (sq_s[b], s_sb[b], rs_s[b]),
                    (sq_t[b], t_sb[b], rs_t[b]),
                ):
                    nc.vector.tensor_mul(out=sq[:], in0=sb[:], in1=sb[:])
                    nc.vector.reduce_sum(
                        out=rs[:], in_=sq[:], axis=mybir.AxisListType.X
                    )
                    nc.vector.tensor_scalar_max(out=rs[:], in0=rs[:], scalar1=EPS)
                    nc.scalar.sqrt(out=rs[:], in_=rs[:])
                    nc.vector.reciprocal(out=rs[:], in_=rs[:])
                nc.vector.tensor_scalar(
                    out=s_sb[b][:], in0=s_sb[b][:], scalar1=rs_s[b][:],
                    scalar2=rs_t[b][:], op0=mult, op1=mult,
                )
                nc.scalar.dma_start(out=s_hat_d[b : b + 1, :], in_=s_sb[b][:])

            # s_row[rt][p, :] = s_hat[b(row)] via partition-stride-0 DMA bcast
            for rt, (r0, plen) in enumerate(ROW_TILES):
                for p0, b, h0, seglen in _segments(r0, plen):
                    base = s_hat_d[b, :]
                    bcast = bass.AP(
                        tensor=base.tensor,
                        offset=base.offset,
                        ap=[[0, seglen]] + list(base.ap),
                    )
                    nc.scalar.dma_start(
                        out=s_row[rt][p0 : p0 + seglen, :], in_=bcast
                    )

            # a[rt] = s_row * t_col + roi (full-width DVE, start partition 0)
            for rt, (r0, plen) in enumerate(ROW_TILES):
                nc.vector.tensor_scalar_mul(
                    out=a_sb[rt][:plen, :], in0=s_row[rt][:plen, :],
                    scalar1=t_col[rt][:plen, :],
                )
                nc.vector.tensor_add(
                    out=a_sb[rt][:plen, :], in0=a_sb[rt][:plen, :],
                    in1=roi_sb[rt][:plen, :],
                )

            # ---- main bandwidth-bound stream ----
            for rt, (r0, plen) in enumerate(ROW_TILES):
                for w0, w1 in W_SPLITS:
                    nw = w1 - w0
                    t = big.tile([128, 7, C], dt, name="stream", tag="stream")
                    to = bigo.tile([128, 7, C], dt, name="ostream", tag="ostream")
                    nc.sync.dma_start(
                        out=t[:plen, :nw, :], in_=ipt[r0 : r0 + plen, w0:w1, :]
                    )
                    for wi in range(nw):
                        nc.vector.tensor_scalar_mul(
                            out=to[:plen, wi, :],
                            in0=t[:plen, wi, :],
                            scalar1=a_sb[rt][:plen, w0 + wi : w0 + wi + 1],
                        )
                    nc.scalar.dma_start(
                        out=out[r0 : r0 + plen, w0:w1, :], in_=to[:plen, :nw, :]
                    )
    nc.finalize()
    return nc


def _get_nc():
    if not _NC_CACHE:
        _NC_CACHE.append(_build())
    return _NC_CACHE[0]


def _make_in_maps(s_o, t_o, ipt, roi_map):
    s_o = np.ascontiguousarray(np.asarray(s_o, dtype=np.float32))
    t_o = np.ascontiguousarray(np.asarray(t_o, dtype=np.float32))
    ipt = np.asarray(ipt, dtype=np.float32)
    roi_map = np.asarray(roi_map, dtype=np.float32)
    roi_rep = np.ascontiguousarray(
        np.broadcast_to(roi_map.reshape(1, H, W), (NB, H, W)).reshape(NR, W)
    )
    in_maps = []
    for i in range(N_CORES):
        lo, hi = i * NB, (i + 1) * NB
        in_maps.append(
            {
                "s_o": s_o[lo:hi],
                "t_o": t_o[lo:hi],
                "ipt": np.ascontiguousarray(ipt[lo:hi]).reshape(NR, W, C),
                "roi_map": roi_rep,
            }
        )
    return in_maps


def _execute(in_maps, **kwargs):
    nc = _get_nc()
    return run_bass_kernel_spmd(nc, in_maps, core_ids=list(range(N_CORES)), **kwargs)


def kernel(s_o, t_o, ipt, roi_map):
    in_maps = _make_in_maps(s_o, t_o, ipt, roi_map)
    res = _execute(in_maps)
    return np.concatenate(
        [res.results[i]["out"].reshape(NB, H, W, C) for i in range(N_CORES)], axis=0
    )


# revision 25
# speedup vs baseline: 1.0894x; 1.0203x over previous
"""Trainium2 Bass kernel for nn_AttentionMapLayer.

Computes out[b,h,w,c] = (l2n(s_o)[b,w] * l2n(t_o)[b,h] + roi[h,w]) * ipt[b,h,w,c]
where l2n is tf-style l2_normalize (x * rsqrt(max(sum(x^2), 1e-12))).

Sharding: pure data parallel over batch (16) across 8 NeuronCores, 2 batches
per core; roi_map replicated. Per core the kernel is HBM-bandwidth bound:
~30.7 MB read (ipt shard) + ~30.7 MB written (out shard).

Per-core structure (v3):
  - (b, h) flattened to 600 rows; ipt/out declared as [600, 25, 512] and
    roi_map replicated host-side to [600, 25], so stream tiles use the full
    128 partitions (=> all 16 SDMA engines) and prologue loads are one DMA
    per row-tile (no batch-boundary segment DMAs except the s broadcast).
  - l2-normalization factors on 1-partition tiles; both rsqrt factors folded
    into s: s_hat = s_o * rs_s * rs_t, so a = s_hat (outer) t_o_raw + roi.
    (A K=1 TensorE matmul outer product wedges the device; tensor_tensor_
    reduce also wedges it — both avoided.)
  - s_hat broadcast across partitions by a partition-stride-0 DMA read from
    a DRAM scratch; t_o loaded column-wise (rows on partitions).
  - a/roi/t/s tiles are separate per row-tile so the Tile framework's
    per-tensor semaphores let row-tile 0's stream start as soon as ITS
    attention row is ready, not after the whole prologue.
  - main stream: row tiles of 128|88 partitions x 9|8-w chunks; DMA in on
    SyncE HWDGE queue (SyncE issues nothing else, so ins start at t=0),
    multiply into a separate out tile, DMA out on ScalarE HWDGE queue
    (ScalarE also issues the small prologue DMAs, which it finishes before
    the first out is ready).
"""

import os
import sys

import numpy as np

for _p in (
    "/root/.axon_site",
    "/root/.axon_site/_ro/trn_rl_repo",
    "/root/.axon_site/_ro/pypackages",
    "/opt/trn_rl_repo",
):
    if os.path.isdir(_p) and _p not in sys.path:
        sys.path.append(_p)

import concourse.bacc as bacc
import concourse.bass as bass
import concourse.tile as tile
from concourse import mybir
from concourse.bass_utils import run_bass_kernel_spmd

N_CORES = 8
B, H, W, C = 16, 300, 25, 512
NB = B // N_CORES   # batches per core
NR = NB * H         # flattened rows per core
ROW_TILES = ((0, 128), (128, 128), (512, 88), (256, 128), (384, 128))
W_SPLITS = ((0, 7), (7, 13), (13, 19), (19, 25))
EPS = 1e-12

_NC_CACHE = []


def _segments(r0, plen):
    """Split rows [r0, r0+plen) at batch boundaries -> (p0, b, h0, seglen)."""
    segs = []
    r = r0
    while r < r0 + plen:
        b, h0 = divmod(r, H)
        seglen = min(r0 + plen - r, H - h0)
        segs.append((r - r0, b, h0, seglen))
        r += seglen
    return segs


def _build():
    dt = mybir.dt.float32
    nc = bacc.Bacc(None)
    s_o = nc.declare_dram_parameter("s_o", [NB, W], dt, isOutput=False)
    t_o = nc.declare_dram_parameter("t_o", [NB, H], dt, isOutput=False)
    ipt = nc.declare_dram_parameter("ipt", [NR, W, C], dt, isOutput=False)
    roi = nc.declare_dram_parameter("roi_map", [NR, W], dt, isOutput=False)
    out = nc.declare_dram_parameter("out", [NR, W, C], dt, isOutput=True)

    t_flat = t_o.rearrange("b h -> (b h)")
    mult = mybir.AluOpType.mult
    NT = len(ROW_TILES)

    with tile.TileContext(nc) as tc:
        with (
            tc.tile_pool(name="small", bufs=1) as small,
            tc.tile_pool(name="dram", bufs=1, space="DRAM") as dram,
            tc.tile_pool(name="big", bufs=6) as big,
            tc.tile_pool(name="early", bufs=2) as early,
            tc.tile_pool(name="bigo", bufs=4) as bigo,
        ):
            s_hat_d = dram.tile([NB, W], dt)
            # per-batch 1-partition tiles for the normalization factors
            s_sb = [small.tile([1, W], dt, name=f"s{b}", tag=f"s{b}") for b in range(NB)]
            t_sb = [small.tile([1, H], dt, name=f"t{b}", tag=f"t{b}") for b in range(NB)]
            sq_s = [small.tile([1, W], dt, name=f"qs{b}", tag=f"qs{b}") for b in range(NB)]
            sq_t = [small.tile([1, H], dt, name=f"qt{b}", tag=f"qt{b}") for b in range(NB)]
            rs_s = [small.tile([1, 1], dt, name=f"rs{b}", tag=f"rs{b}") for b in range(NB)]
            rs_t = [small.tile([1, 1], dt, name=f"rt{b}", tag=f"rt{b}") for b in range(NB)]
            # per-row-tile tiles (separate tensors -> fine-grained semaphores)
            t_col = [small.tile([128, 1], dt, name=f"tc{i}", tag=f"tc{i}") for i in range(NT)]
            roi_sb = [small.tile([128, W], dt, name=f"ro{i}", tag=f"ro{i}") for i in range(NT)]
            s_row = [small.tile([128, W], dt, name=f"sr{i}", tag=f"sr{i}") for i in range(NT)]
            a_sb = [small.tile([128, W], dt, name=f"a{i}", tag=f"a{i}") for i in range(NT)]

            # two early chunk-1 ins on the ScalarE queue from a DEDICATED
            # pool (the SyncE queue alone caps ~250 GB/s during the ramp;
            # these must not consume the main stream pool's slots).
            early_in = {}
            for rt in (0, 1):
                r0, plen = ROW_TILES[rt]
                w0, w1 = W_SPLITS[1]
                te = early.tile([128, 7, C], dt, name="early", tag="early")
                nc.scalar.dma_start(
                    out=te[:plen, : w1 - w0, :], in_=ipt[r0 : r0 + plen, w0:w1, :]
                )
                early_in[rt] = te

            # ---- prologue loads ----
            # rt0-critical chain on ScalarE HWDGE (fast issue; SyncE kept
            # free for stream ins); rt1.. loads on GpSimd (parallel issuer).
            for b in range(NB):
                nc.scalar.dma_start(out=s_sb[b][:], in_=s_o[b : b + 1, :])
                nc.scalar.dma_start(out=t_sb[b][:], in_=t_o[b : b + 1, :])
            for rt, (r0, plen) in enumerate(ROW_TILES):
                nc.scalar.dma_start(
                    out=t_col[rt][:plen, :], in_=t_flat[r0 : r0 + plen]
                )
                nc.scalar.dma_start(
                    out=roi_sb[rt][:plen, :], in_=roi[r0 : r0 + plen, :]
                )

            # rs = 1/sqrt(max(sum(x^2), eps)) per vector; fold both into s:
            # s_hat = s_o * rs_s * rs_t  (so a = s_hat (outer) t_o + roi)
            for b in range(NB):
                for sq, sb, rs in (
                    (sq_s[b], s_sb[b], rs_s[b]),
                    (sq_t[b], t_sb[b], rs_t[b]),
                ):
                    nc.vector.tensor_mul(out=sq[:], in0=sb[:], in1=sb[:])
                    nc.vector.reduce_sum(
                        out=rs[:], in_=sq[:], axis=mybir.AxisListType.X
                    )
                    nc.vector.tensor_scalar_max(out=rs[:], in0=rs[:], scalar1=EPS)
                    nc.scalar.sqrt(out=rs[:], in_=rs[:])
                    nc.vector.reciprocal(out=rs[:], in_=rs[:])
                nc.vector.tensor_scalar(
                    out=s_sb[b][:], in0=s_sb[b][:], scalar1=rs_s[b][:],
                    scalar2=rs_t[b][:], op0=mult, op1=mult,
                )
                nc.scalar.dma_start(out=s_hat_d[b : b + 1, :], in_=s_sb[b][:])

            # s_row[rt][p, :] = s_hat[b(row)] via partition-stride-0 DMA bcast
            for rt, (r0, plen) in enumerate(ROW_TILES):
                for p0, b, h0, seglen in _segments(r0, plen):
                    base = s_hat_d[b, :]
                    bcast = bass.AP(
                        tensor=base.tensor,
                        offset=base.offset,
                        ap=[[0, seglen]] + list(base.ap),
                    )
                    nc.scalar.dma_start(
                        out=s_row[rt][p0 : p0 + seglen, :], in_=bcast
                    )

            # a[rt] = s_row * t_col + roi (full-width DVE, start partition 0)
            for rt, (r0, plen) in enumerate(ROW_TILES):
                nc.vector.tensor_scalar_mul(
                    out=a_sb[rt][:plen, :], in0=s_row[rt][:plen, :],
                    scalar1=t_col[rt][:plen, :],
                )
                nc.vector.tensor_add(
                    out=a_sb[rt][:plen, :], in0=a_sb[rt][:plen, :],
                    in1=roi_sb[rt][:plen, :],
                )

            # ---- main bandwidth-bound stream ----
            for rt, (r0, plen) in enumerate(ROW_TILES):
                for ci, (w0, w1) in enumerate(W_SPLITS):
                    nw = w1 - w0
                    if ci == 1 and rt in early_in:
                        t = early_in[rt]
                    else:
                        t = big.tile([128, 7, C], dt, name="stream", tag="stream")
                        nc.sync.dma_start(
                            out=t[:plen, :nw, :], in_=ipt[r0 : r0 + plen, w0:w1, :]
                        )
                    to = bigo.tile([128, 7, C], dt, name="ostream", tag="ostream")
                    for wi in range(nw):
                        nc.vector.tensor_scalar_mul(
                            out=to[:plen, wi, :],
                            in0=t[:plen, wi, :],
                            scalar1=a_sb[rt][:plen, w0 + wi : w0 + wi + 1],
                        )
                    nc.scalar.dma_start(
                        out=out[r0 : r0 + plen, w0:w1, :], in_=to[:plen, :nw, :]
                    )
    nc.finalize()
    return nc


def _get_nc():
    if not _NC_CACHE:
        _NC_CACHE.append(_build())
    return _NC_CACHE[0]


def _make_in_maps(s_o, t_o, ipt, roi_map):
    s_o = np.ascontiguousarray(np.asarray(s_o, dtype=np.float32))
    t_o = np.ascontiguousarray(np.asarray(t_o, dtype=np.float32))
    ipt = np.asarray(ipt, dtype=np.float32)
    roi_map = np.asarray(roi_map, dtype=np.float32)
    roi_rep = np.ascontiguousarray(
        np.broadcast_to(roi_map.reshape(1, H, W), (NB, H, W)).reshape(NR, W)
    )
    in_maps = []
    for i in range(N_CORES):
        lo, hi = i * NB, (i + 1) * NB
        in_maps.append(
            {
                "s_o": s_o[lo:hi],
                "t_o": t_o[lo:hi],
                "ipt": np.ascontiguousarray(ipt[lo:hi]).reshape(NR, W, C),
                "roi_map": roi_rep,
            }
        )
    return in_maps


def _execute(in_maps, **kwargs):
    nc = _get_nc()
    return run_bass_kernel_spmd(nc, in_maps, core_ids=list(range(N_CORES)), **kwargs)


def kernel(s_o, t_o, ipt, roi_map):
    in_maps = _make_in_maps(s_o, t_o, ipt, roi_map)
    res = _execute(in_maps)
    return np.concatenate(
        [res.results[i]["out"].reshape(NB, H, W, C) for i in range(N_CORES)], axis=0
    )
